# revision 1
# baseline (speedup 1.0000x reference)
"""Complex DFT (512-pt) over rows of x = x_re + i*x_im, y = x @ W^T (complex).

Full inputs: x_re, x_im (8,16,256,512) f32; w_re, w_im (512,512) f32.
Full output: (8,16,256,512,2) f32  (re/im interleaved on last axis).

Strategy: out(m, 2h)=y_re, out(m, 2h+1)=y_im collapses the 4 real matmuls
into ONE (M,1024)@(1024,1024) matmul with an interleaved-column weight
matrix.  Shard batch dim (8) across 8 cores -> per core (4096,1024)@(1024,1024).
PE mapping: psum[m=128, h=512] = lhsT[k=128, m=128].T @ rhs[k=128, h=512],
lhsT = X^T tiles (host-pretiled), rhs = W_big tiles (SBUF-resident).
dtype float32r: full-rate fp32 matmul on trn2 for free-dim >= 256.
"""

import sys

sys.path.insert(0, "/opt/trn_rl_repo")

import numpy as np

import concourse.bass as bass
import concourse.mybir as mybir
import concourse.tile as tile
from concourse import bacc
from concourse.bass_utils import run_bass_kernel_spmd

N = 512          # DFT size
B = 8            # batch -> one per core
M = 4096         # rows per core (16*256)
K = 2 * N        # 1024 contraction (re|im stacked)
H = 2 * N        # 1024 output cols (re/im interleaved)
MT = M // 128    # 32 m-tiles
KT = K // 128    # 8 k-subtiles

_F32 = mybir.dt.float32
_F32R = mybir.dt.float32r


def _build_bass(trace: bool = False):
    # x loads in 1 MB pairs (8 KB/partition descriptors), stores in 2 MB
    # mega-tiles of 4 m-tiles (16 KB/partition descriptors).  Host-side
    # permutes make every descriptor contiguous.
    # Conjugate symmetry of the DFT matrix: W[N-h] = conj(W[h]) means the
    # four real products P1=A@C, P2=B@D, Q1=A@D, Q2=B@C over h=0..256 give
    # BOTH spectrum halves:
    #   y_re[h]=P1-P2, y_im[h]=Q1+Q2, y_re[N-h]=P1+P2, y_im[N-h]=Q2-Q1
    # -> half the matmul columns.  Device writes the four combined slabs
    # contiguously; the host permutes columns into the interleaved order.
    HH = N // 2 + 1  # 257
    HHP = 264      # padded product width (32B-aligned free dim for fp32r MM)
    nc = bacc.Bacc("TRN2", target_bir_lowering=False, debug=False, num_devices=B)
    xt_d = nc.dram_tensor("xt", [MT, 128, KT * 128], _F32R, kind="ExternalInput")
    w_d = nc.dram_tensor("w", [2, 4, 128, HHP], _F32R, kind="ExternalInput")
    out_d = nc.dram_tensor("out", [MT, 128, H], _F32, kind="ExternalOutput")

    with tile.TileContext(nc) as tc:
        with (
            tc.tile_pool(name="wpool", bufs=1) as wpool,
            tc.tile_pool(name="xpool", bufs=13) as xpool,
            tc.tile_pool(name="opool", bufs=16) as opool,
            tc.tile_pool(name="psum", bufs=3, space="PSUM") as pspool,
        ):
            zb = wpool.tile([128, 1], _F32, tag="zb", name="zb")
            nc.gpsimd.memset(zb[:], 0.0)
            cts, dts = [], []
            for k in range(4):
                ct = wpool.tile([128, HHP], _F32R, tag=f"ct{k}", name=f"ct{k}")
                nc.scalar.dma_start(ct[:], w_d[0, k][:])
                cts.append(ct)
            for k in range(4):
                dt = wpool.tile([128, HHP], _F32R, tag=f"dt{k}", name=f"dt{k}")
                nc.scalar.dma_start(dt[:], w_d[1, k][:])
                dts.append(dt)
            for mt in range(MT):
                xs = xpool.tile([128, KT * 128], _F32R, tag="xs")
                nc.sync.dma_start(xs[:], xt_d[mt][:])
                p1 = pspool.tile([128, HHP], _F32, tag="p1", name="p1", bufs=1)
                q1 = pspool.tile([128, HHP], _F32, tag="q1", name="q1", bufs=1)
                p2 = pspool.tile([128, HHP], _F32, tag="p2", name="p2")
                q2 = pspool.tile([128, HHP], _F32, tag="q2", name="q2")
                for ps_t, rhs_t, koff in (
                    (p1, cts, 0),
                    (q1, dts, 0),
                    (p2, dts, 4),
                    (q2, cts, 4),
                ):
                    for k in range(4):
                        nc.tensor.matmul(
                            ps_t[:],
                            xs[:, (koff + k) * 128 : (koff + k + 1) * 128],
                            rhs_t[k][:],
                            start=(k == 0),
                            stop=(k == 3),
                        )
                ot = opool.tile([128, H], _F32, tag="ot")
                # TensorTensor may read only ONE input from PSUM: stage
                # P1/Q1 in SBUF, combine against P2/Q2 still in PSUM.
                t1 = opool.tile([128, HHP], _F32, tag="t1", name="t1")
                t2 = opool.tile([128, HHP], _F32, tag="t2", name="t2")
                # ACT (mostly idle) evacuates P1/Q1 so DVE only runs the
                # four combine ops -> breaks the PE/DVE 71us/71us tie.
                nc.scalar.activation(
                    t1[:], p1[:], mybir.ActivationFunctionType.Copy, bias=0.0
                )
                nc.scalar.activation(
                    t2[:], q1[:], mybir.ActivationFunctionType.Copy, bias=0.0
                )
                _add = mybir.AluOpType.add
                _sub = mybir.AluOpType.subtract
                nc.vector.tensor_tensor(ot[:, 0:HH], t1[:, 0:HH], p2[:, 0:HH], _sub)
                nc.vector.tensor_tensor(
                    ot[:, 2 * HH : 2 * HH + 255], t1[:, 1:256], p2[:, 1:256], _add
                )
                nc.vector.tensor_tensor(ot[:, HH : 2 * HH], t2[:, 0:HH], q2[:, 0:HH], _add)
                nc.vector.tensor_tensor(
                    ot[:, 2 * HH + 255 : H], q2[:, 1:256], t2[:, 1:256], _sub
                )
                # All stores on the idle gpsimd SWDGE queue: a store's
                # event-sem wait (on DVE combines) must not head-of-line
                # block the ACT queue, which runs the PSUM evacuations.
                nc.gpsimd.dma_start(out_d[mt][:], ot[:])
    nc.compile()
    return nc


_cached = {}


def _get_bass(trace=False):
    key = bool(trace)
    if key not in _cached:
        _cached[key] = _build_bass(trace)
    return _cached[key]


_HH = N // 2 + 1


def _perm():
    # final interleaved column -> device slab column
    p = np.empty(H, np.int64)
    for h in range(_HH):
        p[2 * h] = h
        p[2 * h + 1] = _HH + h
    for h in range(1, 256):
        p[2 * (N - h)] = 2 * _HH + h - 1
        p[2 * (N - h) + 1] = 2 * _HH + 255 + h - 1
    return p


_PERM = _perm()


def _prep_weights(w_re, w_im):
    w_re = np.asarray(w_re, np.float32)
    w_im = np.asarray(w_im, np.float32)
    HHP = 264
    w = np.empty((2, 4, 128, HHP), np.float32)
    # ct[k, p, h] = w_re[h, k*128+p]; dt likewise with w_im (padded cols unused)
    w[0] = w_re[:HHP].T.reshape(4, 128, HHP)
    w[1] = w_im[:HHP].T.reshape(4, 128, HHP)
    return np.ascontiguousarray(w)


def _prep_x_core(xr, xi):
    # Xcat = [x_re | x_im] (M, 1024); lhsT tile layout (MT, 128p=k-in-block, KT, 128f=m-in-block)
    xcat_t = np.empty((K, M), np.float32)
    xcat_t[:N] = xr.reshape(M, N).T
    xcat_t[N:] = xi.reshape(M, N).T
    # (K, M) -> per-m-tile lhsT stripes [128p=k-in-block, KT, 128f=m-in-block]
    xt = xcat_t.reshape(KT, 128, MT, 128).transpose(2, 1, 0, 3)
    return np.ascontiguousarray(xt).reshape(MT, 128, KT * 128)


def kernel(x_re, x_im, w_re, w_im, _trace=False, _trace_kwargs=None):
    x_re = np.asarray(x_re, np.float32)
    x_im = np.asarray(x_im, np.float32)
    w_big = _prep_weights(w_re, w_im)
    in_maps = [
        {"xt": _prep_x_core(x_re[c], x_im[c]), "w": w_big} for c in range(B)
    ]
    nc = _get_bass(_trace)
    res = run_bass_kernel_spmd(
        nc, in_maps, list(range(B)), trace=_trace, **(_trace_kwargs or {})
    )
    out = np.empty((B, 16, 256, N, 2), np.float32)
    for c in range(B):
        oc = res.results[c]["out"].reshape(M, H)[:, _PERM]
        out[c] = oc.reshape(16, 256, N, 2)
    if _trace:
        kernel._last_result = res
    return out



# revision 2
# speedup vs baseline: 1.3431x; 1.3431x over previous
"""Complex DFT (512-pt) over rows of x = x_re + i*x_im, y = x @ W^T (complex).

Full inputs: x_re, x_im (8,16,256,512) f32; w_re, w_im (512,512) f32.
Full output: (8,16,256,512,2) f32  (re/im interleaved on last axis).

Strategy (fp16 everywhere on-device; tolerance gate is 2e-2, fp16 lands ~4e-4):
  Conjugate symmetry W[N-h] = conj(W[h]) -> only half the spectrum columns
  are computed.  The four half-spectrum real products collapse into TWO
  512-wide matmuls against ONE shared rhs:
      RHS = [ C(h=0..256) | D(h=1..255) ]   (512 cols = exactly 1 PSUM bank)
      PA  = Xre @ RHS = [ P1(0..256) | Q1(1..255) ]
      PB  = Xim @ RHS = [ Q2(0..256) | P2(1..255) ]
  with C = Re(W), D = Im(W), P1=A@C, P2=B@D, Q1=A@D, Q2=B@C.
      y_re[h]     = P1[h] - P2[h]      y_im[h]     = Q1[h] + Q2[h]
      y_re[N-h]   = P1[h] + P2[h]      y_im[N-h]   = Q2[h] - Q1[h]
  Edge columns h=0 and h=256 are plain (alternating-)row-sums of x; the host
  computes them in f32 (P2[0]=P2[256]=Q1[0]=Q1[256]=0).

  Shard batch dim (8) across 8 cores -> per core (4096,512)x(512,512) x2.
  PE mapping: psum[m=128, 512] += lhsT[k=128, m=128].T @ rhs[k=128, 512],
  fp16 at 1 cycle/row -> ~55us PE per core; fp16 IO halves HBM traffic to
  ~16 MB/core.  ACT evacuates PA/PB (f32 PSUM -> fp16 SBUF), DVE runs the
  four combines at its 2x fp16 rate, gpsimd stores 4-m-tile mega-tiles.
"""

import sys

sys.path.insert(0, "/opt/trn_rl_repo")

import numpy as np

import concourse.bass as bass
import concourse.mybir as mybir
import concourse.tile as tile
from concourse import bacc
from concourse.bass_utils import run_bass_kernel_spmd

N = 512          # DFT size
B = 8            # batch -> one per core
M = 4096         # rows per core (16*256)
K = N            # contraction per product
KT = K // 128    # 4 k-subtiles per product
MT = M // 128    # 32 m-tiles
MP = 2           # m-tiles per input DMA
MS = 4           # m-tiles per output DMA (mega-store)
OW = 1020        # output cols per m-tile (4 slabs of 255)

_F32 = mybir.dt.float32
_F16 = mybir.dt.float16


def _build_bass():
    nc = bacc.Bacc("TRN2", target_bir_lowering=False, debug=False, num_devices=B)
    # xt[i] holds m-tiles 2i, 2i+1: free dim = (half, kblock(8), 128 m)
    # kblocks 0..3 = x_re, 4..7 = x_im (each [128 k-in-block, 128 m] lhsT).
    xt_d = nc.dram_tensor("xt", [MT // MP, 128, MP * 8 * 128], _F16, kind="ExternalInput")
    w_d = nc.dram_tensor("w", [KT, 128, 512], _F16, kind="ExternalInput")
    out_d = nc.dram_tensor("out", [MT // MS, 128, MS * OW], _F16, kind="ExternalOutput")

    _add = mybir.AluOpType.add
    _sub = mybir.AluOpType.subtract
    _copy = mybir.ActivationFunctionType.Copy

    with tile.TileContext(nc) as tc:
        with (
            tc.tile_pool(name="wpool", bufs=1) as wpool,
            tc.tile_pool(name="xpool", bufs=4) as xpool,
            tc.tile_pool(name="tpool", bufs=4) as tpool,
            tc.tile_pool(name="opool", bufs=2) as opool,
            tc.tile_pool(name="psum", bufs=3, space="PSUM") as pspool,
        ):
            ws = []
            for k in range(KT):
                wt = wpool.tile([128, 512], _F16, tag=f"w{k}", name=f"w{k}")
                nc.scalar.dma_start(wt[:], w_d[k][:])
                ws.append(wt)
            for mt2 in range(MT // MP):
                xs = xpool.tile([128, MP * 8 * 128], _F16, tag="xs")
                nc.sync.dma_start(xs[:], xt_d[mt2][:])
                for half in range(MP):
                    mt = mt2 * MP + half
                    if mt % MS == 0:
                        ot = opool.tile([128, MS * OW], _F16, tag="ot")
                    oo = (mt % MS) * OW
                    xb = half * 8 * 128
                    pa = pspool.tile([128, 512], _F32, tag="pa", name="pa")
                    pb = pspool.tile([128, 512], _F32, tag="pb", name="pb")
                    for k in range(KT):
                        nc.tensor.matmul(
                            pa[:],
                            xs[:, xb + k * 128 : xb + (k + 1) * 128],
                            ws[k][:],
                            start=(k == 0),
                            stop=(k == KT - 1),
                        )
                    for k in range(KT):
                        nc.tensor.matmul(
                            pb[:],
                            xs[:, xb + (4 + k) * 128 : xb + (5 + k) * 128],
                            ws[k][:],
                            start=(k == 0),
                            stop=(k == KT - 1),
                        )
                    # ACT evacuates PSUM -> fp16 SBUF so the DVE combines run
                    # all-SBUF all-fp16 (2x DVE rate, one-PSUM-operand rule moot)
                    ta = tpool.tile([128, 512], _F16, tag="ta", name="ta")
                    tb = tpool.tile([128, 512], _F16, tag="tb", name="tb")
                    nc.scalar.activation(ta[:], pa[:], _copy, bias=0.0)
                    nc.scalar.activation(tb[:], pb[:], _copy, bias=0.0)
                    # slabs: [y_re 1..255 | y_im 1..255 | y_re N-1..257 rev | y_im rev]
                    nc.vector.tensor_tensor(
                        ot[:, oo : oo + 255], ta[:, 1:256], tb[:, 257:512], _sub
                    )
                    nc.vector.tensor_tensor(
                        ot[:, oo + 255 : oo + 510], ta[:, 257:512], tb[:, 1:256], _add
                    )
                    nc.vector.tensor_tensor(
                        ot[:, oo + 510 : oo + 765], ta[:, 1:256], tb[:, 257:512], _add
                    )
                    nc.vector.tensor_tensor(
                        ot[:, oo + 765 : oo + 1020], tb[:, 1:256], ta[:, 257:512], _sub
                    )
                    if mt % MS == MS - 1:
                        nc.gpsimd.dma_start(out_d[mt // MS][:], ot[:])
    nc.compile()
    return nc


_cached = {}


def _get_bass(trace=False):
    if "nc" not in _cached:
        _cached["nc"] = _build_bass()
    return _cached["nc"]


def _prep_weights(w_re, w_im):
    w_re = np.asarray(w_re, np.float32)
    w_im = np.asarray(w_im, np.float32)
    rhs = np.empty((K, 512), np.float16)
    rhs[:, 0:257] = w_re[0:257].T
    rhs[:, 257:512] = w_im[1:256].T
    return np.ascontiguousarray(rhs.reshape(KT, 128, 512))


def _prep_x_core(xr, xi):
    # lhsT: xcat_t[k, m] with k = [re 512 | im 512]; tile to
    # (MT/2 pairs, 128 k-in-block, (half, kblock, 128 m))
    xcat_t = np.empty((2 * N, M), np.float16)
    xcat_t[:N] = xr.reshape(M, N).T
    xcat_t[N:] = xi.reshape(M, N).T
    xt = xcat_t.reshape(8, 128, MT // MP, MP, 128).transpose(2, 1, 3, 0, 4)
    return np.ascontiguousarray(xt).reshape(MT // MP, 128, MP * 8 * 128)


def kernel(x_re, x_im, w_re, w_im, _trace=False, _trace_kwargs=None):
    x_re = np.asarray(x_re, np.float32)
    x_im = np.asarray(x_im, np.float32)
    w = _prep_weights(w_re, w_im)
    in_maps = [
        {"xt": _prep_x_core(x_re[c], x_im[c]), "w": w} for c in range(B)
    ]
    nc = _get_bass(_trace)
    res = run_bass_kernel_spmd(
        nc, in_maps, list(range(B)), trace=_trace, **(_trace_kwargs or {})
    )
    # edge spectrum cols (h=0, 256) in full f32 on host: plain/alternating row sums
    alt = np.empty(N, np.float32)
    alt[0::2] = 1.0
    alt[1::2] = -1.0
    e0_re = x_re.sum(-1)                  # (B,16,256)
    e0_im = x_im.sum(-1)
    e256_re = x_re @ alt
    e256_im = x_im @ alt
    out = np.empty((B, 16, 256, N, 2), np.float32)
    for c in range(B):
        slab = (
            res.results[c]["out"]
            .reshape(MT // MS, 128, MS, OW)
            .transpose(0, 2, 1, 3)
            .reshape(M, OW)
            .astype(np.float32)
        )
        y = np.empty((M, N, 2), np.float32)
        y[:, 1:256, 0] = slab[:, 0:255]
        y[:, 1:256, 1] = slab[:, 255:510]
        y[:, 257:512, 0] = slab[:, 510:765][:, ::-1]
        y[:, 257:512, 1] = slab[:, 765:1020][:, ::-1]
        y[:, 0, 0] = e0_re[c].ravel()
        y[:, 0, 1] = e0_im[c].ravel()
        y[:, 256, 0] = e256_re[c].ravel()
        y[:, 256, 1] = e256_im[c].ravel()
        out[c] = y.reshape(16, 256, N, 2)
    if _trace:
        kernel._last_result = res
    return out
